# revision 12
# baseline (speedup 1.0000x reference)
"""VQ codebook kernel (nn_NaiveCodebook) for 8 TRN2 NeuronCores — bf16 v5.

Math (per batch row r):
    x   = (img1 - img2) @ W_in                      (b_in cancels in x1-x2)
    d2k = ||x||^2 - 2<x, b_k> + ||b_k||^2
    norm_res = sqrt(min_k d2k)
    scale = norm_res / ||rand|| + eps
    out = (x + scale * rand) @ W_out + b_out

All HBM streams are bf16, host pre-tiled so each DMA is 128 partitions
x one contiguous run.  Host-side work is layout only (transpose / cast /
tiling) plus constant-folding ||b_k||^2/2 from the codebook weights.

Device pipeline per core (rows = 512):
  A: stream img1^T/img2^T (interleaved) + W_in; diff on DVE; accumulate
     x^T = W_in^T @ diff^T into 4 PSUM banks (K=12288).  ||rand||^2 and
     its sqrt/reciprocal chain run here too (they have no deps).
  B: stream book^T; per 512-code tile: -c2/2 seeded into PSUM via a K=1
     ones (x) (-c2/2) matmul, then G accumulates; one DVE
     tensor_reduce(max) per tile into a column of m_all; one final
     reduce collapses the columns.
  S: ||x||^2 via ACT Square + ones-matmul partition sums; ns2 =
     -2*max + ||x||^2 (one fused op); scale = sqrt(ns2)/||rand|| + eps;
     broadcast via K=1 ones (x) scale matmul; q^T = x^T + s*r^T.
  C: stream W_out; paired 2-bank PSUM tiles; one DVE add (+bias bcast
     tile) per 1024 columns writes bf16 out tiles; DMA out.
"""

import os
import sys

for _p in (
    "/root/.axon_site",
    "/root/.axon_site/_ro/trn_rl_repo",
    "/opt/trn_rl_repo",
):
    if os.path.isdir(_p) and _p not in sys.path:
        sys.path.append(_p)

import numpy as np
import ml_dtypes

import concourse.bacc as bacc
import concourse.bass as bass
import concourse.tile as tile
from concourse import mybir
from concourse.bass_utils import run_bass_kernel_spmd

F32 = mybir.dt.float32
BF16 = mybir.dt.bfloat16
ALU = mybir.AluOpType
ACTF = mybir.ActivationFunctionType
BF = ml_dtypes.bfloat16

B, C_, H_, W_ = 4096, 3, 64, 64
IN_DIM = C_ * H_ * W_  # 12288
EMB = 512
K = 8192
EPS = 1e-6
NCORES = 8
P = 128
KB = 8


def build_program(rows=B // NCORES, in_dim=IN_DIM, emb=EMB, k=K, kb=KB):
    """Single-core Bass program (SPMD across 8 cores)."""
    assert rows % P == 0 and emb % P == 0 and in_dim % (P * kb) == 0
    assert k % 1024 == 0 and in_dim % 1024 == 0
    mch = rows // P           # row chunks (4)
    ech = emb // P            # emb chunks (4)
    nkb = in_dim // (P * kb)  # phase-A DMA batches (12)
    nd = k // 512             # codebook 512-tiles (16)
    ndp = nd // 2             # paired book DMAs (8)
    no = in_dim // 512        # output column tiles (24)
    nop = no // 2             # paired w_out DMAs / paired PSUM tiles (12)

    nc = bacc.Bacc()
    img12T = nc.declare_dram_parameter(
        "img12T", [nkb, P, kb, 2, rows], BF16, isOutput=False)
    w_in = nc.declare_dram_parameter(
        "w_in", [nkb, P, kb, emb], BF16, isOutput=False)
    bookT = nc.declare_dram_parameter(
        "bookT", [ndp, P, ech, 1024], BF16, isOutput=False)
    c2 = nc.declare_dram_parameter("c2", [1, k], F32, isOutput=False)
    randT = nc.declare_dram_parameter("randT", [P, ech, rows], BF16, isOutput=False)
    w_out = nc.declare_dram_parameter(
        "w_out", [nop, P, ech, 1024], BF16, isOutput=False)
    b_out = nc.declare_dram_parameter("b_out", [1, in_dim], BF16, isOutput=False)
    out = nc.declare_dram_parameter("out", [rows, in_dim], BF16, isOutput=True)

    def bcast_ap(handle, count):
        ap = handle.ap()
        return bass.AP(
            tensor=ap.tensor,
            offset=ap.offset,
            ap=[[0, count]] + list(ap.ap)[1:],
        )

    with tile.TileContext(nc) as tc:
        with tc.tile_pool(name="persist", bufs=1) as persist:
            xT = persist.tile([P, ech, rows], BF16, tag="xT")
            rT = persist.tile([P, ech, rows], BF16, tag="rT")
            qT = persist.tile([P, ech, rows], BF16, tag="qT")
            m_all = [
                persist.tile([P, ndp], F32, tag=f"ma{m}", name=f"ma{m}")
                for m in range(mch)
            ]
            m_fin = [
                persist.tile([P, 1], F32, tag=f"mf{m}", name=f"mf{m}")
                for m in range(mch)
            ]
            ones_k = persist.tile([P, 1], BF16, tag="ones_k")   # K=128 sum lhsT
            ones_m = persist.tile([1, P], BF16, tag="ones_m")   # K=1 fold/bcast lhsT
            ones_mf = persist.tile([1, P], F32, tag="ones_mf")  # f32 K=1 (warm-PE)
            c2n = persist.tile([1, k], BF16, tag="c2n")         # -||b||^2/2
            bbb = persist.tile([P, in_dim], BF16, tag="bbb")    # bias bcast
            mT = persist.tile([1, rows], F32, tag="mT")
            ns2 = persist.tile([1, rows], F32, tag="ns2")
            nres = persist.tile([1, rows], F32, tag="nres")
            nrnd = persist.tile([1, rows], F32, tag="nrnd")
            nrir = persist.tile([1, rows], F32, tag="nrir")
            scT = persist.tile([1, rows], F32, tag="scT")
            scb = persist.tile([1, rows], BF16, tag="scb")
            qtm = persist.tile([P, rows], F32, tag="qtm")
            sq = [
                persist.tile([P, rows], BF16, tag=f"sq{i}", name=f"sq{i}")
                for i in range(2)
            ]
            nc.vector.memset(ones_k, 1.0)
            nc.vector.memset(ones_m, 1.0)
            nc.vector.memset(ones_mf, 1.0)
            nc.gpsimd.dma_start(out=c2n, in_=c2.ap())   # f32 -> bf16 cast DMA
            nc.vector.tensor_scalar_mul(c2n, c2n, -1.0)
            nc.sync.dma_start(out=rT, in_=randT.ap())

            # ---------------- Phase A: xT = W_in^T @ diff^T ----------------
            with (
                tc.tile_pool(name="astream", bufs=2) as ast,
                tc.tile_pool(name="psum_a", bufs=1, space="PSUM") as psa,
            ):
                px = [
                    psa.tile([P, rows], F32, tag=f"px{e}", name=f"px{e}")
                    for e in range(ech)
                ]
                ps_nr = psa.tile([1, rows], F32, tag="pnr", name="ps_nr")
                # ||rand||^2 chain — no deps; hides under A's DMA-bound stretch
                for e in range(ech):
                    nc.scalar.activation(sq[e % 2], rT[:, e, :], ACTF.Square)
                    nc.tensor.matmul(
                        ps_nr, lhsT=ones_k, rhs=sq[e % 2],
                        start=(e == 0), stop=(e == ech - 1),
                    )
                nc.scalar.sqrt(nrnd, ps_nr[0:1, :])
                nc.vector.reciprocal(nrir, nrnd)
                for n in range(nkb):
                    t12 = ast.tile([P, kb, 2, rows], BF16, tag="t12")
                    wt = ast.tile([P, kb, emb], BF16, tag="wi")
                    dt = ast.tile([P, kb, rows], BF16, tag="dt")
                    nc.sync.dma_start(out=t12, in_=img12T.ap()[n])
                    nc.sync.dma_start(out=wt, in_=w_in.ap()[n])
                    nc.vector.tensor_sub(dt, t12[:, :, 0, :], t12[:, :, 1, :])
                    for j in range(kb):
                        for e in range(ech):
                            nc.tensor.matmul(
                                px[e],
                                lhsT=wt[:, j, e * P : (e + 1) * P],
                                rhs=dt[:, j, :],
                                start=(n == 0 and j == 0),
                                stop=(n == nkb - 1 and j == kb - 1),
                            )
                for e in range(ech):
                    if e % 2 == 0:
                        nc.vector.tensor_copy(xT[:, e, :], px[e])
                    else:
                        nc.scalar.copy(xT[:, e, :], px[e])


            # -------- Phase B: running max_k (G - c2/2) -------------------
            # cstream/couts open first so w_out prefetch + bbb land in SBUF
            # space disjoint from the book stream (no WAR on B's matmuls).
            outap = out.ap()
            with (
                tc.tile_pool(name="cstream", bufs=3) as cst,
                tc.tile_pool(name="couts", bufs=2) as cout,
            ):
                nc.sync.dma_start(out=bbb, in_=bcast_ap(b_out, P))
                with (
                    tc.tile_pool(name="bstream", bufs=3) as bst,
                    tc.tile_pool(name="bscratch", bufs=2) as bscr,
                    tc.tile_pool(name="psum_b", bufs=3, space="PSUM") as psb,
                    tc.tile_pool(name="psum_s", bufs=1, space="PSUM") as pss,
                ):
                    ps_sx = pss.tile([1, rows], F32, tag="psx", name="ps_sx")
                    ps_sc = pss.tile([P, rows], F32, tag="psc", name="ps_sc")
                    for bp in range(ndp):
                        bt = bst.tile([P, ech, 1024], BF16, tag="bt")
                        nc.sync.dma_start(out=bt, in_=bookT.ap()[bp])
                        for m in range(mch):
                            # paired 2-bank PSUM tile: halves share one xT
                            # weight load per e and one reduce per 1024 codes
                            ps2 = psb.tile([P, 1024], F32, tag="d")
                            for t in range(2):
                                nc.tensor.matmul(
                                    ps2[:, t * 512 : (t + 1) * 512],
                                    lhsT=ones_m,
                                    rhs=c2n[0:1, (bp * 2 + t) * 512 : (bp * 2 + t + 1) * 512],
                                    start=True,
                                    stop=False,
                                )
                            for e in range(ech):
                                for t in range(2):
                                    nc.tensor.matmul(
                                        ps2[:, t * 512 : (t + 1) * 512],
                                        lhsT=xT[:, e, m * P : (m + 1) * P],
                                        rhs=bt[:, e, t * 512 : (t + 1) * 512],
                                        start=False,
                                        stop=(e == ech - 1),
                                    )
                            nc.vector.tensor_reduce(
                                m_all[m][:, bp : bp + 1],
                                ps2,
                                axis=mybir.AxisListType.X,
                                op=ALU.max,
                            )
                    for m in range(mch):
                        nc.vector.tensor_reduce(
                            m_fin[m], m_all[m], axis=mybir.AxisListType.X,
                            op=ALU.max,
                        )

                    # ---------- Phase S: per-row scalars + quant^T ----------
                    for e in range(ech):
                        nc.scalar.activation(sq[e % 2], xT[:, e, :], ACTF.Square)
                        nc.tensor.matmul(
                            ps_sx, lhsT=ones_k, rhs=sq[e % 2],
                            start=(e == 0), stop=(e == ech - 1),
                        )
                    # mT[0, m*P + p] = m_fin[m][p]  (partition -> free)
                    for m in range(mch):
                        nc.sync.dma_start(
                            out=mT[0:1, m * P : (m + 1) * P], in_=m_fin[m]
                        )
                    # ns2 = ||x||^2 - 2*max = min_k d2; scale chain in bf16
                    nc.vector.scalar_tensor_tensor(
                        out=ns2, in0=mT, scalar=-2.0, in1=ps_sx[0:1, :],
                        op0=ALU.mult, op1=ALU.add,
                    )
                    nc.scalar.sqrt(nres, ns2)
                    nc.vector.tensor_mul(scT, nres, nrir)
                    nc.vector.tensor_scalar_add(scb, scT, EPS)
                    # broadcast: ps_sc = ones (x) scale;  qT = xT + ps_sc * rT
                    nc.tensor.matmul(
                        ps_sc, lhsT=ones_m, rhs=scb, start=True, stop=True
                    )
                    for e in range(ech):
                        nc.vector.tensor_mul(qtm, rT[:, e, :], ps_sc)
                        nc.vector.tensor_add(qT[:, e, :], xT[:, e, :], qtm)

                # ---- Phase C: out = quant @ W_out + b_out ----
                with tc.tile_pool(name="psum_c", bufs=2, space="PSUM") as psc:
                    osb = None
                    for gg in range(nop):
                        wt = cst.tile([P, ech, 1024], BF16, tag="wo")
                        nc.sync.dma_start(out=wt, in_=w_out.ap()[gg])
                        if gg % 2 == 0:
                            osb = [
                                cout.tile([P, 2, 1024], BF16, tag=f"osb{m}", name=f"osb{m}")
                                for m in range(mch)
                            ]
                        last = gg == nop - 1
                        for m in range(mch):
                            ps2 = psc.tile([P, 1024], F32, tag="o")
                            for t in range(2):
                                if last:
                                    nc.tensor.matmul(
                                        ps2[:, t * 512 : (t + 1) * 512],
                                        lhsT=ones_m,
                                        rhs=bbb[0:1, gg * 1024 + t * 512 : gg * 1024 + (t + 1) * 512],
                                        start=True,
                                        stop=False,
                                    )
                                for e in range(ech):
                                    nc.tensor.matmul(
                                        ps2[:, t * 512 : (t + 1) * 512],
                                        lhsT=qT[:, e, m * P : (m + 1) * P],
                                        rhs=wt[:, e, t * 512 : (t + 1) * 512],
                                        start=(e == 0 and not last),
                                        stop=(e == ech - 1),
                                    )
                            if last:
                                if m % 2 == 0:
                                    nc.vector.tensor_copy(osb[m][:, gg % 2, :], ps2)
                                else:
                                    nc.scalar.copy(osb[m][:, gg % 2, :], ps2)
                            else:
                                nc.vector.tensor_add(
                                    osb[m][:, gg % 2, :],
                                    ps2,
                                    bbb[:, gg * 1024 : (gg + 1) * 1024],
                                )
                        if gg % 2 == 1:
                            g = gg // 2
                            for m in range(mch):
                                nc.sync.dma_start(
                                    out=outap[
                                        m * P : (m + 1) * P,
                                        g * 2048 : (g + 1) * 2048,
                                    ],
                                    in_=osb[m],
                                )
    nc.finalize()
    return nc


def make_shards(image_1, image_2, random_vector, W_in, b_in, W_out, b_out, book,
                rows=None, ncores=NCORES, kb=KB):
    x1 = np.asarray(image_1, np.float32).reshape(np.shape(image_1)[0], -1)
    x2 = np.asarray(image_2, np.float32).reshape(np.shape(image_2)[0], -1)
    rv = np.asarray(random_vector, np.float32)
    nrows_total = x1.shape[0]
    if rows is None:
        rows = nrows_total // ncores
    in_dim = x1.shape[1]
    emb = np.shape(W_in)[1]
    k = np.shape(book)[0]
    ech = emb // P
    nkb = in_dim // (P * kb)
    ndp = k // 1024
    nop = in_dim // 1024

    x1b = x1.astype(BF)
    x2b = x2.astype(BF)
    w_in_b = np.asarray(W_in, np.float32).astype(BF)
    # w_in tiled: [nkb, P, kb, emb];  row index d = (n*kb + j)*P + p
    w_in_t = np.ascontiguousarray(
        w_in_b.reshape(nkb, kb, P, emb).transpose(0, 2, 1, 3)
    )
    bookT_b = np.asarray(book, np.float32).astype(BF).T  # [emb, k]
    bookT_t = np.ascontiguousarray(
        bookT_b.reshape(ech, P, ndp, 1024).transpose(2, 1, 0, 3)
    )
    c2_c = (np.sum(np.asarray(book, np.float64) ** 2, axis=1) / 2.0).astype(
        np.float32
    ).reshape(1, k)
    w_out_b = np.asarray(W_out, np.float32).astype(BF)  # [emb, in_dim]
    w_out_t = np.ascontiguousarray(
        w_out_b.reshape(ech, P, nop, 1024).transpose(2, 1, 0, 3)
    )
    b_out_c = np.ascontiguousarray(
        np.asarray(b_out, np.float32).astype(BF)
    ).reshape(1, in_dim)
    shards = []
    for i in range(ncores):
        sl = slice(i * rows, (i + 1) * rows)
        # img12 tiled: [nkb, P, kb, 2, rows]; d = (n*kb + j)*P + p
        img12 = np.empty((nkb, P, kb, 2, rows), BF)
        img12[:, :, :, 0, :] = (
            x1b[sl].T.reshape(nkb, kb, P, rows).transpose(0, 2, 1, 3)
        )
        img12[:, :, :, 1, :] = (
            x2b[sl].T.reshape(nkb, kb, P, rows).transpose(0, 2, 1, 3)
        )
        randT_t = np.ascontiguousarray(
            rv[sl].T.astype(BF).reshape(ech, P, rows).transpose(1, 0, 2)
        )
        shards.append(
            {
                "img12T": img12,
                "w_in": w_in_t,
                "bookT": bookT_t,
                "c2": c2_c,
                "randT": randT_t,
                "w_out": w_out_t,
                "b_out": b_out_c,
            }
        )
    return shards


_prog_cache = {}


def _get_program():
    if "nc" not in _prog_cache:
        _prog_cache["nc"] = build_program()
    return _prog_cache["nc"]


def run(inputs, trace=False):
    """Run on the 8 NeuronCores; returns (full_output, BassKernelResults)."""
    nc = _get_program()
    shards = make_shards(**inputs)
    res = run_bass_kernel_spmd(nc, shards, core_ids=list(range(NCORES)), trace=trace)
    out = np.concatenate(
        [np.asarray(res.results[i]["out"], np.float32) for i in range(NCORES)],
        axis=0,
    )
    return out, res


def kernel(**inputs):
    out, _ = run(inputs, trace=False)
    return out


# revision 13
# speedup vs baseline: 1.1019x; 1.1019x over previous
"""VQ codebook kernel (nn_NaiveCodebook) for 8 TRN2 NeuronCores — bf16 v5.

Math (per batch row r):
    x   = (img1 - img2) @ W_in                      (b_in cancels in x1-x2)
    d2k = ||x||^2 - 2<x, b_k> + ||b_k||^2
    norm_res = sqrt(min_k d2k)
    scale = norm_res / ||rand|| + eps
    out = (x + scale * rand) @ W_out + b_out

All HBM streams are bf16, host pre-tiled so each DMA is 128 partitions
x one contiguous run.  Host-side work is layout only (transpose / cast /
tiling) plus constant-folding ||b_k||^2/2 from the codebook weights.

Device pipeline per core (rows = 512):
  A: stream img1^T/img2^T (interleaved) + W_in; diff on DVE; accumulate
     x^T = W_in^T @ diff^T into 4 PSUM banks (K=12288).  ||rand||^2 and
     its sqrt/reciprocal chain run here too (they have no deps).
  B: stream book^T; per 512-code tile: -c2/2 seeded into PSUM via a K=1
     ones (x) (-c2/2) matmul, then G accumulates; one DVE
     tensor_reduce(max) per tile into a column of m_all; one final
     reduce collapses the columns.
  S: ||x||^2 via ACT Square + ones-matmul partition sums; ns2 =
     -2*max + ||x||^2 (one fused op); scale = sqrt(ns2)/||rand|| + eps;
     broadcast via K=1 ones (x) scale matmul; q^T = x^T + s*r^T.
  C: stream W_out; paired 2-bank PSUM tiles; one DVE add (+bias bcast
     tile) per 1024 columns writes bf16 out tiles; DMA out.
"""

import os
import sys

for _p in (
    "/root/.axon_site",
    "/root/.axon_site/_ro/trn_rl_repo",
    "/opt/trn_rl_repo",
):
    if os.path.isdir(_p) and _p not in sys.path:
        sys.path.append(_p)

import numpy as np
import ml_dtypes

import concourse.bacc as bacc
import concourse.bass as bass
import concourse.tile as tile
from concourse import mybir
from concourse.bass_utils import run_bass_kernel_spmd

F32 = mybir.dt.float32
BF16 = mybir.dt.bfloat16
ALU = mybir.AluOpType
ACTF = mybir.ActivationFunctionType
BF = ml_dtypes.bfloat16

B, C_, H_, W_ = 4096, 3, 64, 64
IN_DIM = C_ * H_ * W_  # 12288
EMB = 512
K = 8192
EPS = 1e-6
NCORES = 8
P = 128
KB = 8


def build_program(rows=B // NCORES, in_dim=IN_DIM, emb=EMB, k=K, kb=KB):
    """Single-core Bass program (SPMD across 8 cores)."""
    assert rows % P == 0 and emb % P == 0 and in_dim % (P * kb) == 0
    assert k % 1024 == 0 and in_dim % 1024 == 0
    mch = rows // P           # row chunks (4)
    ech = emb // P            # emb chunks (4)
    nkb = in_dim // (P * kb)  # phase-A DMA batches (12)
    nd = k // 512             # codebook 512-tiles (16)
    ndp = nd // 2             # paired book DMAs (8)
    no = in_dim // 512        # output column tiles (24)
    nop = no // 2             # paired w_out DMAs / paired PSUM tiles (12)

    nc = bacc.Bacc()
    img12T = nc.declare_dram_parameter(
        "img12T", [nkb, P, kb, 2, rows], BF16, isOutput=False)
    w_in = nc.declare_dram_parameter(
        "w_in", [nkb, P, kb, emb], BF16, isOutput=False)
    bookT = nc.declare_dram_parameter(
        "bookT", [ndp, P, ech, 1024], BF16, isOutput=False)
    c2 = nc.declare_dram_parameter("c2", [1, k], F32, isOutput=False)
    randT = nc.declare_dram_parameter("randT", [P, ech, rows], BF16, isOutput=False)
    w_out = nc.declare_dram_parameter(
        "w_out", [nop, P, ech, 1024], BF16, isOutput=False)
    b_out = nc.declare_dram_parameter("b_out", [1, in_dim], BF16, isOutput=False)
    out = nc.declare_dram_parameter("out", [rows, in_dim], BF16, isOutput=True)

    def bcast_ap(handle, count):
        ap = handle.ap()
        return bass.AP(
            tensor=ap.tensor,
            offset=ap.offset,
            ap=[[0, count]] + list(ap.ap)[1:],
        )

    with tile.TileContext(nc) as tc:
        with tc.tile_pool(name="persist", bufs=1) as persist:
            xT = persist.tile([P, ech, rows], BF16, tag="xT")
            rT = persist.tile([P, ech, rows], BF16, tag="rT")
            qT = persist.tile([P, ech, rows], BF16, tag="qT")
            m_all = [
                persist.tile([P, nd], F32, tag=f"ma{m}", name=f"ma{m}")
                for m in range(mch)
            ]
            m_fin = [
                persist.tile([P, 1], F32, tag=f"mf{m}", name=f"mf{m}")
                for m in range(mch)
            ]
            ones_k = persist.tile([P, 1], BF16, tag="ones_k")   # K=128 sum lhsT
            ones_m = persist.tile([1, P], BF16, tag="ones_m")   # K=1 fold/bcast lhsT
            ones_mf = persist.tile([1, P], F32, tag="ones_mf")  # f32 K=1 (warm-PE)
            c2n = persist.tile([1, k], BF16, tag="c2n")         # -||b||^2/2
            bbb = persist.tile([P, in_dim], BF16, tag="bbb")    # bias bcast
            mT = persist.tile([1, rows], F32, tag="mT")
            ns2 = persist.tile([1, rows], F32, tag="ns2")
            nres = persist.tile([1, rows], F32, tag="nres")
            nrnd = persist.tile([1, rows], F32, tag="nrnd")
            nrir = persist.tile([1, rows], F32, tag="nrir")
            scT = persist.tile([1, rows], F32, tag="scT")
            scb = persist.tile([1, rows], BF16, tag="scb")
            qtm = persist.tile([P, rows], F32, tag="qtm")
            sq = [
                persist.tile([P, rows], BF16, tag=f"sq{i}", name=f"sq{i}")
                for i in range(2)
            ]
            nc.vector.memset(ones_k, 1.0)
            nc.vector.memset(ones_m, 1.0)
            nc.vector.memset(ones_mf, 1.0)
            nc.gpsimd.dma_start(out=c2n, in_=c2.ap())   # f32 -> bf16 cast DMA
            nc.vector.tensor_scalar_mul(c2n, c2n, -1.0)
            nc.sync.dma_start(out=rT, in_=randT.ap())

            # ---------------- Phase A: xT = W_in^T @ diff^T ----------------
            with (
                tc.tile_pool(name="astream", bufs=2) as ast,
                tc.tile_pool(name="psum_a", bufs=1, space="PSUM") as psa,
            ):
                px = [
                    psa.tile([P, rows], F32, tag=f"px{e}", name=f"px{e}")
                    for e in range(ech)
                ]
                ps_nr = psa.tile([1, rows], F32, tag="pnr", name="ps_nr")
                # ||rand||^2 chain — no deps; hides under A's DMA-bound stretch
                for e in range(ech):
                    nc.scalar.activation(sq[e % 2], rT[:, e, :], ACTF.Square)
                    nc.tensor.matmul(
                        ps_nr, lhsT=ones_k, rhs=sq[e % 2],
                        start=(e == 0), stop=(e == ech - 1),
                    )
                nc.scalar.sqrt(nrnd, ps_nr[0:1, :])
                nc.vector.reciprocal(nrir, nrnd)
                for n in range(nkb):
                    t12 = ast.tile([P, kb, 2, rows], BF16, tag="t12")
                    wt = ast.tile([P, kb, emb], BF16, tag="wi")
                    dt = ast.tile([P, kb, rows], BF16, tag="dt")
                    nc.sync.dma_start(out=t12, in_=img12T.ap()[n])
                    nc.sync.dma_start(out=wt, in_=w_in.ap()[n])
                    nc.vector.tensor_sub(dt, t12[:, :, 0, :], t12[:, :, 1, :])
                    for j in range(kb):
                        for e in range(ech):
                            nc.tensor.matmul(
                                px[e],
                                lhsT=wt[:, j, e * P : (e + 1) * P],
                                rhs=dt[:, j, :],
                                start=(n == 0 and j == 0),
                                stop=(n == nkb - 1 and j == kb - 1),
                            )
                for e in range(ech):
                    if e % 2 == 0:
                        nc.vector.tensor_copy(xT[:, e, :], px[e])
                    else:
                        nc.scalar.copy(xT[:, e, :], px[e])


            # -------- Phase B: running max_k (G - c2/2) -------------------
            # cstream/couts open first so w_out prefetch + bbb land in SBUF
            # space disjoint from the book stream (no WAR on B's matmuls).
            outap = out.ap()
            with (
                tc.tile_pool(name="cstream", bufs=3) as cst,
                tc.tile_pool(name="couts", bufs=2) as cout,
            ):
                nc.sync.dma_start(out=bbb, in_=bcast_ap(b_out, P))
                with (
                    tc.tile_pool(name="bstream", bufs=3) as bst,
                    tc.tile_pool(name="bscratch", bufs=2) as bscr,
                    tc.tile_pool(name="psum_b", bufs=6, space="PSUM") as psb,
                    tc.tile_pool(name="psum_s", bufs=1, space="PSUM") as pss,
                ):
                    ps_sx = pss.tile([1, rows], F32, tag="psx", name="ps_sx")
                    ps_sc = pss.tile([P, rows], F32, tag="psc", name="ps_sc")
                    for bp in range(ndp):
                        bt = bst.tile([P, ech, 1024], BF16, tag="bt")
                        nc.sync.dma_start(out=bt, in_=bookT.ap()[bp])
                        for t in range(2):
                            n = bp * 2 + t
                            for m in range(mch):
                                ps = psb.tile([P, 512], F32, tag="d")
                                nc.tensor.matmul(
                                    ps,
                                    lhsT=ones_m,
                                    rhs=c2n[0:1, n * 512 : (n + 1) * 512],
                                    start=True,
                                    stop=False,
                                )
                                for e in range(ech):
                                    nc.tensor.matmul(
                                        ps,
                                        lhsT=xT[:, e, m * P : (m + 1) * P],
                                        rhs=bt[:, e, t * 512 : (t + 1) * 512],
                                        start=False,
                                        stop=(e == ech - 1),
                                    )
                                nc.vector.tensor_reduce(
                                    m_all[m][:, n : n + 1],
                                    ps,
                                    axis=mybir.AxisListType.X,
                                    op=ALU.max,
                                )
                    for m in range(mch):
                        nc.vector.tensor_reduce(
                            m_fin[m], m_all[m], axis=mybir.AxisListType.X,
                            op=ALU.max,
                        )

                    # ---------- Phase S: per-row scalars + quant^T ----------
                    for e in range(ech):
                        nc.scalar.activation(sq[e % 2], xT[:, e, :], ACTF.Square)
                        nc.tensor.matmul(
                            ps_sx, lhsT=ones_k, rhs=sq[e % 2],
                            start=(e == 0), stop=(e == ech - 1),
                        )
                    # mT[0, m*P + p] = m_fin[m][p]  (partition -> free)
                    for m in range(mch):
                        nc.sync.dma_start(
                            out=mT[0:1, m * P : (m + 1) * P], in_=m_fin[m]
                        )
                    # ns2 = ||x||^2 - 2*max = min_k d2; scale chain in bf16
                    nc.vector.scalar_tensor_tensor(
                        out=ns2, in0=mT, scalar=-2.0, in1=ps_sx[0:1, :],
                        op0=ALU.mult, op1=ALU.add,
                    )
                    nc.scalar.sqrt(nres, ns2)
                    nc.vector.tensor_mul(scT, nres, nrir)
                    nc.vector.tensor_scalar_add(scb, scT, EPS)
                    # broadcast: ps_sc = ones (x) scale;  qT = xT + ps_sc * rT
                    nc.tensor.matmul(
                        ps_sc, lhsT=ones_m, rhs=scb, start=True, stop=True
                    )
                    for e in range(ech):
                        nc.vector.tensor_mul(qtm, rT[:, e, :], ps_sc)
                        nc.vector.tensor_add(qT[:, e, :], xT[:, e, :], qtm)

                # ---- Phase C: out = quant @ W_out + b_out ----
                with tc.tile_pool(name="psum_c", bufs=2, space="PSUM") as psc:
                    osb = None
                    for gg in range(nop):
                        wt = cst.tile([P, ech, 1024], BF16, tag="wo")
                        nc.sync.dma_start(out=wt, in_=w_out.ap()[gg])
                        if gg % 2 == 0:
                            osb = [
                                cout.tile([P, 2, 1024], BF16, tag=f"osb{m}", name=f"osb{m}")
                                for m in range(mch)
                            ]
                        last = gg == nop - 1
                        for m in range(mch):
                            ps2 = psc.tile([P, 1024], F32, tag="o")
                            for t in range(2):
                                if last:
                                    nc.tensor.matmul(
                                        ps2[:, t * 512 : (t + 1) * 512],
                                        lhsT=ones_m,
                                        rhs=bbb[0:1, gg * 1024 + t * 512 : gg * 1024 + (t + 1) * 512],
                                        start=True,
                                        stop=False,
                                    )
                                for e in range(ech):
                                    nc.tensor.matmul(
                                        ps2[:, t * 512 : (t + 1) * 512],
                                        lhsT=qT[:, e, m * P : (m + 1) * P],
                                        rhs=wt[:, e, t * 512 : (t + 1) * 512],
                                        start=(e == 0 and not last),
                                        stop=(e == ech - 1),
                                    )
                            if last:
                                if m % 2 == 0:
                                    nc.vector.tensor_copy(osb[m][:, gg % 2, :], ps2)
                                else:
                                    nc.scalar.copy(osb[m][:, gg % 2, :], ps2)
                            else:
                                nc.vector.tensor_add(
                                    osb[m][:, gg % 2, :],
                                    ps2,
                                    bbb[:, gg * 1024 : (gg + 1) * 1024],
                                )
                        if gg % 2 == 1:
                            g = gg // 2
                            for m in range(mch):
                                nc.sync.dma_start(
                                    out=outap[
                                        m * P : (m + 1) * P,
                                        g * 2048 : (g + 1) * 2048,
                                    ],
                                    in_=osb[m],
                                )
    nc.finalize()
    return nc


def make_shards(image_1, image_2, random_vector, W_in, b_in, W_out, b_out, book,
                rows=None, ncores=NCORES, kb=KB):
    x1 = np.asarray(image_1, np.float32).reshape(np.shape(image_1)[0], -1)
    x2 = np.asarray(image_2, np.float32).reshape(np.shape(image_2)[0], -1)
    rv = np.asarray(random_vector, np.float32)
    nrows_total = x1.shape[0]
    if rows is None:
        rows = nrows_total // ncores
    in_dim = x1.shape[1]
    emb = np.shape(W_in)[1]
    k = np.shape(book)[0]
    ech = emb // P
    nkb = in_dim // (P * kb)
    ndp = k // 1024
    nop = in_dim // 1024

    x1b = x1.astype(BF)
    x2b = x2.astype(BF)
    w_in_b = np.asarray(W_in, np.float32).astype(BF)
    # w_in tiled: [nkb, P, kb, emb];  row index d = (n*kb + j)*P + p
    w_in_t = np.ascontiguousarray(
        w_in_b.reshape(nkb, kb, P, emb).transpose(0, 2, 1, 3)
    )
    bookT_b = np.asarray(book, np.float32).astype(BF).T  # [emb, k]
    bookT_t = np.ascontiguousarray(
        bookT_b.reshape(ech, P, ndp, 1024).transpose(2, 1, 0, 3)
    )
    c2_c = (np.sum(np.asarray(book, np.float64) ** 2, axis=1) / 2.0).astype(
        np.float32
    ).reshape(1, k)
    w_out_b = np.asarray(W_out, np.float32).astype(BF)  # [emb, in_dim]
    w_out_t = np.ascontiguousarray(
        w_out_b.reshape(ech, P, nop, 1024).transpose(2, 1, 0, 3)
    )
    b_out_c = np.ascontiguousarray(
        np.asarray(b_out, np.float32).astype(BF)
    ).reshape(1, in_dim)
    shards = []
    for i in range(ncores):
        sl = slice(i * rows, (i + 1) * rows)
        # img12 tiled: [nkb, P, kb, 2, rows]; d = (n*kb + j)*P + p
        img12 = np.empty((nkb, P, kb, 2, rows), BF)
        img12[:, :, :, 0, :] = (
            x1b[sl].T.reshape(nkb, kb, P, rows).transpose(0, 2, 1, 3)
        )
        img12[:, :, :, 1, :] = (
            x2b[sl].T.reshape(nkb, kb, P, rows).transpose(0, 2, 1, 3)
        )
        randT_t = np.ascontiguousarray(
            rv[sl].T.astype(BF).reshape(ech, P, rows).transpose(1, 0, 2)
        )
        shards.append(
            {
                "img12T": img12,
                "w_in": w_in_t,
                "bookT": bookT_t,
                "c2": c2_c,
                "randT": randT_t,
                "w_out": w_out_t,
                "b_out": b_out_c,
            }
        )
    return shards


_prog_cache = {}


def _get_program():
    if "nc" not in _prog_cache:
        _prog_cache["nc"] = build_program()
    return _prog_cache["nc"]


def run(inputs, trace=False):
    """Run on the 8 NeuronCores; returns (full_output, BassKernelResults)."""
    nc = _get_program()
    shards = make_shards(**inputs)
    res = run_bass_kernel_spmd(nc, shards, core_ids=list(range(NCORES)), trace=trace)
    out = np.concatenate(
        [np.asarray(res.results[i]["out"], np.float32) for i in range(NCORES)],
        axis=0,
    )
    return out, res


def kernel(**inputs):
    out, _ = run(inputs, trace=False)
    return out
